# revision 8
# baseline (speedup 1.0000x reference)
"""Trainium2 Bass kernel for a spiking MLP (IF neurons, soft reset) — v4.

Count-form reformulation. For a soft-reset IF neuron, with
X(t) = cumulative input and c(t) = cumulative spike count:

    spike(t)  =  X(t) >= 1 + c(t-1)
    c(t)      =  c(t-1) + spike(t)

and, critically, the next layer's cumulative input is itself a matmul
of the COUNTS:  X_{l+1}(t) = W_l @ c_l(t)  (counts are small exact ints
in fp16). So every timestep does FRESH matmuls on counts (start/stop
per step — no persistent PSUM integrators, no cross-step PSUM
serialization), and the per-layer elementwise work collapses into ONE
custom fused DVE op per layer:

    c' = c + ((X * s0) >= c + 1)        [SPIKE_COUNT_UPDATE]

(s0 = t+1 for layer 1, whose cumulative input is h1*(t+1); 1.0 for
layers 2/3). Layer 4 runs on the COUNTS (u(t) = w4 @ c3(t), the two
contract chunks concurrent in PE column groups 0/32); the exact
temporal diff out_t = u(t) - u(t-1) happens on host during unsharding.

Exactness: weights stream as fp16 hi/lo pairs (residual ~2^-22|w|) —
the 2-stream cost is the information floor (fp32r is 11-bit/TF32-like,
fp8 lo-terms flush to denormals, and the margin-noise budget is ~2e-6).

v4 over v3: (1) h1 = w1 @ x collapses its three contract-16 fp16 hi/lo
terms into ONE contract-48 matmul per m-chunk ([w1h; w1l; w1h] @
[xh; xh; xl] stacked along the contract dim); (2) the t-loop pipeline
is deepened — L3 runs 2 steps behind L2, L4 three behind, with DVE
issue order [c3(i-2), c1(i+1), c2(i)] so the single-buffered X3 bank
recycles without stalling the PE and the L2 group never waits on its
c1; (3) per-block xS DMA slices + block-0's first c1 reading h1
straight from PSUM shave the startup ramp. Measured: PE 95.3% busy
with ~5.5us true idle; ~525us vs the 532us v3 baseline.

PSUM: X2 double-buffered (4 banks) + X3 (2) + out double-buffered (2) = 8.
"""

import numpy as np

import concourse.bacc as bacc
import concourse.mybir as mybir
from concourse.tile import TileContext
from concourse import bass_utils

T = 16
B = 32768
NCORES = 8
BL = B // NCORES          # 4096 rows per core
BLK = 512                 # batch columns per block
D = 256                   # hidden width
KC = D // 128             # 2 feature chunks of 128
W = 2 * BLK               # paired-chunk tile width
NBLK = BL // BLK

F32 = mybir.dt.float32
FP16 = mybir.dt.float16

_CACHE = {}


def _register_count_op():
    """Register the fused spike-count-update DVE op:
        out = in1 + ((in0 * s0) >= in1 + 1)
    One pass over X and c producing the new count (spike folded in)."""
    from concourse import dve_ops
    from concourse.dve_spec import (
        Spec, Src0, Src1, One, lower as dve_lower, _has_src1)
    from concourse.dve_uop import DveOpSpec

    name = "SPIKE_COUNT_UPDATE_ANT"
    for op in dve_ops.OPS:
        if op.name == name:
            return op
    from concourse.dve_spec import C0
    spec = Spec(
        body=Src1 + ((Src0 * C0) >= (Src1 + One)),
        reference=lambda in0, in1, c0, c1, c2: in1
        + ((in0 * c0) >= (in1 + 1.0)),
    )
    row = dve_ops._CUSTOM_DVE_ROW_BASE + len(dve_ops.OPS)
    assert row < 0x20, "custom-DVE opcode rows exhausted"
    dve_ops._SUB_OPCODE_FOR_NAME[name] = row
    shas = {}
    for ver in ("v3", "v4"):
        try:
            uops = dve_lower(spec, ver=ver)
            shas[ver] = DveOpSpec(
                name=name, opcode=row, uops=uops,
                rd1_en=_has_src1(spec)).sha(ver)
        except Exception:
            pass
    op = dve_ops.DveOp(name, spec, subdim=False, uops_sha=shas)
    dve_ops.OPS.append(op)
    dve_ops.CUSTOM_DVE_SPECS[name] = spec
    return op


def _build_nc(bl=BL):
    op_count = _register_count_op()
    nc = bacc.Bacc("TRN2", target_bir_lowering=False, debug=False,
                   num_devices=NCORES)

    # h1 = w1h@xh + w1l@xh + w1h@xl: the three contract-16 terms stack
    # along the contract dim into ONE contract-48 matmul:
    # rows 0:16 = (w1h, xh), 16:32 = (w1l, xh), 32:48 = (w1h, xl).
    xS_d = nc.dram_tensor("xS", [48, bl], FP16, kind="ExternalInput")
    w1S_d = nc.dram_tensor("w1S", [48, D], FP16, kind="ExternalInput")
    # packed lhsT chunks, fp16 hi/lo:
    # [128, (k*2+m)*128 : ...] = w.T[k*128:(k+1)*128, m*128:(m+1)*128]
    w2h_d = nc.dram_tensor("w2h", [128, 512], FP16, kind="ExternalInput")
    w2l_d = nc.dram_tensor("w2l", [128, 512], FP16, kind="ExternalInput")
    w3h_d = nc.dram_tensor("w3h", [128, 512], FP16, kind="ExternalInput")
    w3l_d = nc.dram_tensor("w3l", [128, 512], FP16, kind="ExternalInput")
    # [128, k*16:(k+1)*16] = w4.T[k*128:(k+1)*128, :]
    w4h_d = nc.dram_tensor("w4h", [128, 32], FP16, kind="ExternalInput")
    out_d = nc.dram_tensor("out", [T, 48, bl], F32, kind="ExternalOutput")

    with TileContext(nc) as tc:
        with (
            tc.tile_pool(name="const", bufs=1) as cpool,
            tc.tile_pool(name="cnt", bufs=3) as npool,
            tc.tile_pool(name="stage", bufs=2) as stpool,
            tc.tile_pool(name="x2p", bufs=2, space="PSUM") as x2pool,
            tc.tile_pool(name="x3p", bufs=1, space="PSUM") as x3pool,
            tc.tile_pool(name="ops", bufs=2, space="PSUM") as opool,
        ):
            # ---- constants into SBUF ----
            w1S = cpool.tile([48, D], FP16, tag="w1S", name="w1S")
            nc.sync.dma_start(out=w1S[:], in_=w1S_d.ap())
            # per-block xS slices so stage_h1(0) starts after 1/8 of the DMA
            xS = cpool.tile([48, bl], FP16, tag="xS", name="xS")
            for _b in range(NBLK):
                _cs = slice(_b * BLK, (_b + 1) * BLK)
                nc.sync.dma_start(out=xS[:, _cs], in_=xS_d.ap()[:, _cs])
            wt = {}
            for name, dram in (("2h", w2h_d), ("2l", w2l_d),
                               ("3h", w3h_d), ("3l", w3l_d)):
                tl = cpool.tile([128, 512], FP16, tag=f"w{name}",
                                name=f"w{name}")
                nc.sync.dma_start(out=tl[:], in_=dram.ap())
                wt[name] = tl
            w4h = cpool.tile([128, 32], FP16, tag="w4h", name="w4h")
            nc.sync.dma_start(out=w4h[:], in_=w4h_d.ap())
            zero16 = cpool.tile([128, W], FP16, tag="zero16", name="zero16")
            nc.vector.memset(zero16[:], 0.0)

            # ---- h1 = w1 @ x for all blocks, staged through the X2 pool ----
            h1all = cpool.tile([128, NBLK * W], F32, tag="h1all",
                               name="h1all")
            def stage_h1(b):
                """h1 block b = (w1h+w1l) @ (xh+xl), minus the ~2^-22
                l*l term — one contract-48 matmul per m-chunk (the three
                contract-16 terms stacked along the contract dim)."""
                cs = slice(b * BLK, (b + 1) * BLK)
                hps = x2pool.tile([128, W], F32, tag="X2", name="X2")
                for m in range(KC):
                    ms = slice(m * 128, (m + 1) * 128)
                    nc.tensor.matmul(hps[:, m * BLK:(m + 1) * BLK],
                                     w1S[:, ms], xS[:, cs],
                                     start=True, stop=True)
                nc.scalar.copy(out=h1all[:, b * W:(b + 1) * W], in_=hps[:])
                return hps

            # block 0's h1 goes FIRST on the PE queue (it gates the whole
            # t-loop; it runs cold but ~2us earlier than behind the
            # warm-ups). The HAM warm-up matmuls then raise the PE clock
            # to 2.4 GHz while block 0's h1-copy and first count op run.
            h1ps0 = stage_h1(0)
            warm = x3pool.tile([128, W], F32, tag="X3", name="X3")
            for wi in range(4):
                nc.tensor.matmul(warm[:, 0:BLK], zero16[:, 0:128],
                                 zero16[:, 0:BLK], start=True, stop=True)
            if NBLK > 1:
                # block 1's h1 fills the PE gap while DVE/ACT finish
                # block 0's h1 copy and first count op
                stage_h1(1)

            # Software pipeline (per block): L3 runs 2 steps behind L2, L4
            # runs 3 behind.  DVE issue order per iteration is
            # [c3(i-2), c1(i+1), c2(i)]: c3(i-2) reads the X3 produced by
            # the FIRST PE group of this iteration, so it finishes well
            # before the next iteration's L3 group needs the
            # (single-buffered) X3 bank back; c1 for the NEXT step is
            # computed an iteration ahead so the L2 group never waits on
            # it; and c2(i) — which must wait for this iteration's L2
            # matmuls — sits last so it never blocks ready DVE work.
            for b in range(NBLK):
                cs = slice(b * BLK, (b + 1) * BLK)
                h1b = h1all[:, b * W:(b + 1) * W]
                stage = stpool.tile([48, T * BLK], F32, tag="stage",
                                    name="stage")
                c2 = c3 = zero16

                def c1_op(step, c1_state, in0=None):
                    c1n = npool.tile([128, W], FP16, tag="c1", name="c1")
                    nc.vector._custom_dve(op_count, out=c1n[:],
                                          in0=h1b if in0 is None else in0,
                                          in1=c1_state[:],
                                          s0=float(step + 1))
                    return c1n

                # c1 for step 0, issued up front; block 0 reads h1 straight
                # from its PSUM staging tile instead of waiting for the ACT
                # copy into h1all.
                c1_cur = c1_op(0, zero16,
                               in0=h1ps0[:] if b == 0 else None)
                c2_by_t = {}
                c3_by_t = {}
                for i in range(T + 3):
                    if 2 <= i < T + 2:
                        t3 = i - 2
                        # -- L3 matmuls for step t3: X3 = w3 @ c2(t3) --
                        X3 = x3pool.tile([128, W], F32, tag="X3", name="X3")
                        c2t = c2_by_t.pop(t3)
                        for m in range(KC):
                            for j_i, (k, part) in enumerate(
                                    ((0, "3h"), (0, "3l"),
                                     (1, "3h"), (1, "3l"))):
                                j = (k * 2 + m) * 128
                                nc.tensor.matmul(
                                    X3[:, m * BLK:(m + 1) * BLK],
                                    wt[part][:, j:j + 128],
                                    c2t[:, k * BLK:(k + 1) * BLK],
                                    start=(j_i == 0), stop=(j_i == 3))
                        c3n = npool.tile([128, W], FP16, tag="c3", name="c3")
                        nc.vector._custom_dve(op_count, out=c3n[:],
                                              in0=X3[:], in1=c3[:], s0=1.0)
                        c3 = c3n
                        c3_by_t[t3] = c3n
                    if i + 1 < T:
                        # -- L1 count for the NEXT step, an iteration early
                        c1_nxt = c1_op(i + 1, c1_cur)
                    if i < T:
                        # -- L2 matmuls: X2 = w2 @ c1(i) (fresh) --
                        X2 = x2pool.tile([128, W], F32, tag="X2", name="X2")
                        for m in range(KC):
                            for j_i, (k, part) in enumerate(
                                    ((0, "2h"), (0, "2l"),
                                     (1, "2h"), (1, "2l"))):
                                j = (k * 2 + m) * 128
                                nc.tensor.matmul(
                                    X2[:, m * BLK:(m + 1) * BLK],
                                    wt[part][:, j:j + 128],
                                    c1_cur[:, k * BLK:(k + 1) * BLK],
                                    start=(j_i == 0), stop=(j_i == 3))
                        c2n = npool.tile([128, W], FP16, tag="c2", name="c2")
                        nc.vector._custom_dve(op_count, out=c2n[:],
                                              in0=X2[:], in1=c2[:], s0=1.0)
                        c2 = c2n
                        if i + 1 < T:
                            c1_cur = c1_nxt
                        c2_by_t[i] = c2n
                    if i == 2 and b + 2 < NBLK:
                        stage_h1(b + 2)
                    if 3 <= i < T + 3:
                        t4 = i - 3
                        # -- L4 on counts: u(t) = w4 @ c3(t); the two
                        #    contract chunks run CONCURRENTLY in PE column
                        #    groups 0 and 32 (host sums the partition
                        #    slices during unsharding) --
                        c3t = c3_by_t.pop(t4)
                        o = opool.tile([48, BLK], F32, tag="o", name="o")
                        for k in range(KC):
                            nc.tensor.matmul(
                                o[32 * k:32 * k + 16, :],
                                w4h[:, k * 16:(k + 1) * 16],
                                c3t[:, k * BLK:(k + 1) * BLK],
                                start=True, stop=True,
                                tile_position=(0, 32 * k))
                        nc.scalar.copy(out=stage[:, t4 * BLK:(t4 + 1) * BLK],
                                       in_=o[:])
                        nc.sync.dma_start(out=out_d.ap()[t4, :, cs],
                                          in_=stage[:, t4 * BLK:(t4 + 1) * BLK])

    nc.compile()
    return nc


def _prep_inputs(x, w1, w2, w3, w4):
    """Host-side data prep: shard x, transpose/split weights (positive)."""
    def split(a):
        hi = a.astype(np.float16)
        lo = (a.astype(np.float32) - hi.astype(np.float32)).astype(np.float16)
        return np.ascontiguousarray(hi), np.ascontiguousarray(lo)

    def pack_256(wn):  # wn = w.T [256, 256] -> [128, 512] packed (k, m)
        out = np.empty((128, 512), np.float32)
        for k in range(2):
            for m in range(2):
                out[:, (k * 2 + m) * 128:(k * 2 + m + 1) * 128] = \
                    wn[k * 128:(k + 1) * 128, m * 128:(m + 1) * 128]
        return out

    def pack_16(wn):  # wn = w4.T [256, 16] -> [128, 32]
        out = np.empty((128, 32), np.float32)
        for k in range(2):
            out[:, k * 16:(k + 1) * 16] = wn[k * 128:(k + 1) * 128, :]
        return out

    w1hT, w1lT = split(w1.astype(np.float32).T)                  # [16,256]
    w2h, w2l = split(pack_256(w2.astype(np.float32).T))
    w3h, w3l = split(pack_256(w3.astype(np.float32).T))
    w4h = np.ascontiguousarray(
        pack_16(w4.astype(np.float32).T).astype(np.float16))
    w1S = np.zeros((48, 256), np.float16)
    w1S[0:16] = w1hT
    w1S[16:32] = w1lT
    w1S[32:48] = w1hT

    in_maps = []
    for c in range(NCORES):
        xT = np.ascontiguousarray(
            x[c * BL:(c + 1) * BL, :].astype(np.float32).T)      # [16, BL]
        xTh, xTl = split(xT)
        xS = np.zeros((48, xT.shape[1]), np.float16)
        xS[0:16] = xTh
        xS[16:32] = xTh
        xS[32:48] = xTl
        in_maps.append({
            "xS": xS, "w1S": w1S,
            "w2h": w2h, "w2l": w2l,
            "w3h": w3h, "w3l": w3l,
            "w4h": w4h,
        })
    return in_maps


def _get_nc():
    if "nc" not in _CACHE:
        _CACHE["nc"] = _build_nc()
    return _CACHE["nc"]


def run_sharded(x, w1, w2, w3, w4, **spmd_kwargs):
    """Run on 8 cores; returns (full_output, BassKernelResults)."""
    nc = _get_nc()
    in_maps = _prep_inputs(x, w1, w2, w3, w4)
    res = bass_utils.run_bass_kernel_spmd(
        nc, in_maps, core_ids=list(range(NCORES)), **spmd_kwargs)
    # per-core out holds the two col-group halves of u(t) = w4 @ c3(t)
    # in partition rows 0:16 and 32:48; out_t = u(t) - u(t-1).
    parts = []
    for r in res.results:
        u = (r["out"][:, 0:16].astype(np.float32)
             + r["out"][:, 32:48].astype(np.float32))
        u[1:] -= u[:-1].copy()
        parts.append(np.ascontiguousarray(u.transpose(0, 2, 1)))
    full = np.concatenate(parts, axis=1)
    return full, res


def kernel(x, w1, w2, w3, w4):
    full, _ = run_sharded(x, w1, w2, w3, w4)
    return full



# revision 9
# speedup vs baseline: 1.0028x; 1.0028x over previous
"""Trainium2 Bass kernel for a spiking MLP (IF neurons, soft reset) — v4.

Count-form reformulation. For a soft-reset IF neuron, with
X(t) = cumulative input and c(t) = cumulative spike count:

    spike(t)  =  X(t) >= 1 + c(t-1)
    c(t)      =  c(t-1) + spike(t)

and, critically, the next layer's cumulative input is itself a matmul
of the COUNTS:  X_{l+1}(t) = W_l @ c_l(t)  (counts are small exact ints
in fp16). So every timestep does FRESH matmuls on counts (start/stop
per step — no persistent PSUM integrators, no cross-step PSUM
serialization), and the per-layer elementwise work collapses into ONE
custom fused DVE op per layer:

    c' = c + ((X * s0) >= c + 1)        [SPIKE_COUNT_UPDATE]

(s0 = t+1 for layer 1, whose cumulative input is h1*(t+1); 1.0 for
layers 2/3). Layer 4 runs on the COUNTS (u(t) = w4 @ c3(t), the two
contract chunks concurrent in PE column groups 0/32); the exact
temporal diff out_t = u(t) - u(t-1) happens on host during unsharding.

Exactness: weights stream as fp16 hi/lo pairs (residual ~2^-22|w|) —
the 2-stream cost is the information floor (fp32r is 11-bit/TF32-like,
fp8 lo-terms flush to denormals, and the margin-noise budget is ~2e-6).

v4 over v3: (1) h1 = w1 @ x collapses its three contract-16 fp16 hi/lo
terms into ONE contract-48 matmul per m-chunk ([w1h; w1l; w1h] @
[xh; xh; xl] stacked along the contract dim); (2) the t-loop pipeline
is deepened — L3 runs 2 steps behind L2, L4 three behind, with DVE
issue order [c3(i-2), c1(i+1), c2(i)] so the single-buffered X3 bank
recycles without stalling the PE and the L2 group never waits on its
c1; (3) per-block xS DMA slices + block-0's first c1 reading h1
straight from PSUM shave the startup ramp. Measured: PE 95.3% busy
with ~5.5us true idle; ~525us vs the 532us v3 baseline.

PSUM: X2 double-buffered (4 banks) + X3 (2) + out double-buffered (2) = 8.
"""

import numpy as np

import concourse.bacc as bacc
import concourse.mybir as mybir
from concourse.tile import TileContext
from concourse import bass_utils

T = 16
B = 32768
NCORES = 8
BL = B // NCORES          # 4096 rows per core
BLK = 512                 # batch columns per block
D = 256                   # hidden width
KC = D // 128             # 2 feature chunks of 128
W = 2 * BLK               # paired-chunk tile width
NBLK = BL // BLK

F32 = mybir.dt.float32
FP16 = mybir.dt.float16

_CACHE = {}


def _register_count_op():
    """Register the fused spike-count-update DVE op:
        out = in1 + ((in0 * s0) >= in1 + 1)
    One pass over X and c producing the new count (spike folded in)."""
    from concourse import dve_ops
    from concourse.dve_spec import (
        Spec, Src0, Src1, One, lower as dve_lower, _has_src1)
    from concourse.dve_uop import DveOpSpec

    name = "SPIKE_COUNT_UPDATE_ANT"
    for op in dve_ops.OPS:
        if op.name == name:
            return op
    from concourse.dve_spec import C0
    spec = Spec(
        body=Src1 + ((Src0 * C0) >= (Src1 + One)),
        reference=lambda in0, in1, c0, c1, c2: in1
        + ((in0 * c0) >= (in1 + 1.0)),
    )
    row = dve_ops._CUSTOM_DVE_ROW_BASE + len(dve_ops.OPS)
    assert row < 0x20, "custom-DVE opcode rows exhausted"
    dve_ops._SUB_OPCODE_FOR_NAME[name] = row
    shas = {}
    for ver in ("v3", "v4"):
        try:
            uops = dve_lower(spec, ver=ver)
            shas[ver] = DveOpSpec(
                name=name, opcode=row, uops=uops,
                rd1_en=_has_src1(spec)).sha(ver)
        except Exception:
            pass
    op = dve_ops.DveOp(name, spec, subdim=False, uops_sha=shas)
    dve_ops.OPS.append(op)
    dve_ops.CUSTOM_DVE_SPECS[name] = spec
    return op


def _build_nc(bl=BL):
    op_count = _register_count_op()
    nc = bacc.Bacc("TRN2", target_bir_lowering=False, debug=False,
                   num_devices=NCORES)

    # h1 = w1h@xh + w1l@xh + w1h@xl: the three contract-16 terms stack
    # along the contract dim into ONE contract-48 matmul:
    # rows 0:16 = (w1h, xh), 16:32 = (w1l, xh), 32:48 = (w1h, xl).
    xS_d = nc.dram_tensor("xS", [48, bl], FP16, kind="ExternalInput")
    w1S_d = nc.dram_tensor("w1S", [48, D], FP16, kind="ExternalInput")
    # packed lhsT chunks, fp16 hi/lo:
    # [128, (k*2+m)*128 : ...] = w.T[k*128:(k+1)*128, m*128:(m+1)*128]
    w2h_d = nc.dram_tensor("w2h", [128, 512], FP16, kind="ExternalInput")
    w2l_d = nc.dram_tensor("w2l", [128, 512], FP16, kind="ExternalInput")
    w3h_d = nc.dram_tensor("w3h", [128, 512], FP16, kind="ExternalInput")
    w3l_d = nc.dram_tensor("w3l", [128, 512], FP16, kind="ExternalInput")
    # [128, k*16:(k+1)*16] = w4.T[k*128:(k+1)*128, :]
    w4h_d = nc.dram_tensor("w4h", [128, 32], FP16, kind="ExternalInput")
    out_d = nc.dram_tensor("out", [T, 48, bl], F32, kind="ExternalOutput")

    with TileContext(nc) as tc:
        with (
            tc.tile_pool(name="const", bufs=1) as cpool,
            tc.tile_pool(name="cnt", bufs=3) as npool,
            tc.tile_pool(name="stage", bufs=2) as stpool,
            tc.tile_pool(name="x2p", bufs=2, space="PSUM") as x2pool,
            tc.tile_pool(name="x3p", bufs=1, space="PSUM") as x3pool,
            tc.tile_pool(name="ops", bufs=2, space="PSUM") as opool,
        ):
            # ---- constants into SBUF ----
            w1S = cpool.tile([48, D], FP16, tag="w1S", name="w1S")
            nc.sync.dma_start(out=w1S[:], in_=w1S_d.ap())
            # per-block xS slices so stage_h1(0) starts after 1/8 of the DMA
            xS = cpool.tile([48, bl], FP16, tag="xS", name="xS")
            for _b in range(NBLK):
                _cs = slice(_b * BLK, (_b + 1) * BLK)
                nc.sync.dma_start(out=xS[:, _cs], in_=xS_d.ap()[:, _cs])
            wt = {}
            for name, dram in (("2h", w2h_d), ("2l", w2l_d),
                               ("3h", w3h_d), ("3l", w3l_d)):
                tl = cpool.tile([128, 512], FP16, tag=f"w{name}",
                                name=f"w{name}")
                nc.sync.dma_start(out=tl[:], in_=dram.ap())
                wt[name] = tl
            w4h = cpool.tile([128, 32], FP16, tag="w4h", name="w4h")
            nc.sync.dma_start(out=w4h[:], in_=w4h_d.ap())
            zero16 = cpool.tile([128, W], FP16, tag="zero16", name="zero16")
            nc.vector.memset(zero16[:], 0.0)

            # ---- h1 = w1 @ x for all blocks, staged through the X2 pool ----
            h1all = cpool.tile([128, NBLK * W], F32, tag="h1all",
                               name="h1all")
            def stage_h1(b):
                """h1 block b = (w1h+w1l) @ (xh+xl), minus the ~2^-22
                l*l term — one contract-48 matmul per m-chunk (the three
                contract-16 terms stacked along the contract dim)."""
                cs = slice(b * BLK, (b + 1) * BLK)
                hps = x2pool.tile([128, W], F32, tag="X2", name="X2")
                for m in range(KC):
                    ms = slice(m * 128, (m + 1) * 128)
                    nc.tensor.matmul(hps[:, m * BLK:(m + 1) * BLK],
                                     w1S[:, ms], xS[:, cs],
                                     start=True, stop=True)
                nc.scalar.copy(out=h1all[:, b * W:(b + 1) * W], in_=hps[:])
                return hps

            # block 0's h1 goes FIRST on the PE queue (it gates the whole
            # t-loop; it runs cold but ~2us earlier than behind the
            # warm-ups). The HAM warm-up matmuls then raise the PE clock
            # to 2.4 GHz while block 0's h1-copy and first count op run.
            h1ps0 = stage_h1(0)
            warm = x3pool.tile([128, W], F32, tag="X3", name="X3")
            for wi in range(10):
                nc.tensor.matmul(warm[:, 0:BLK], zero16[:, 0:128],
                                 zero16[:, 0:BLK], start=True, stop=True)
            if NBLK > 1:
                # block 1's h1 fills the PE gap while DVE/ACT finish
                # block 0's h1 copy and first count op
                stage_h1(1)

            # Software pipeline (per block): L3 runs 2 steps behind L2, L4
            # runs 3 behind.  DVE issue order per iteration is
            # [c3(i-2), c1(i+1), c2(i)]: c3(i-2) reads the X3 produced by
            # the FIRST PE group of this iteration, so it finishes well
            # before the next iteration's L3 group needs the
            # (single-buffered) X3 bank back; c1 for the NEXT step is
            # computed an iteration ahead so the L2 group never waits on
            # it; and c2(i) — which must wait for this iteration's L2
            # matmuls — sits last so it never blocks ready DVE work.
            for b in range(NBLK):
                cs = slice(b * BLK, (b + 1) * BLK)
                h1b = h1all[:, b * W:(b + 1) * W]
                stage = stpool.tile([48, T * BLK], F32, tag="stage",
                                    name="stage")
                c2 = c3 = zero16

                def c1_op(step, c1_state, in0=None):
                    c1n = npool.tile([128, W], FP16, tag="c1", name="c1")
                    nc.vector._custom_dve(op_count, out=c1n[:],
                                          in0=h1b if in0 is None else in0,
                                          in1=c1_state[:],
                                          s0=float(step + 1))
                    return c1n

                # c1 for step 0, issued up front; block 0 reads h1 straight
                # from its PSUM staging tile instead of waiting for the ACT
                # copy into h1all.
                c1_cur = c1_op(0, zero16,
                               in0=h1ps0[:] if b == 0 else None)
                c2_by_t = {}
                c3_by_t = {}
                for i in range(T + 3):
                    if 2 <= i < T + 2:
                        t3 = i - 2
                        # -- L3 matmuls for step t3: X3 = w3 @ c2(t3) --
                        X3 = x3pool.tile([128, W], F32, tag="X3", name="X3")
                        c2t = c2_by_t.pop(t3)
                        for m in range(KC):
                            for j_i, (k, part) in enumerate(
                                    ((0, "3h"), (0, "3l"),
                                     (1, "3h"), (1, "3l"))):
                                j = (k * 2 + m) * 128
                                nc.tensor.matmul(
                                    X3[:, m * BLK:(m + 1) * BLK],
                                    wt[part][:, j:j + 128],
                                    c2t[:, k * BLK:(k + 1) * BLK],
                                    start=(j_i == 0), stop=(j_i == 3))
                        c3n = npool.tile([128, W], FP16, tag="c3", name="c3")
                        nc.vector._custom_dve(op_count, out=c3n[:],
                                              in0=X3[:], in1=c3[:], s0=1.0)
                        c3 = c3n
                        c3_by_t[t3] = c3n
                    if i + 1 < T:
                        # -- L1 count for the NEXT step, an iteration early
                        c1_nxt = c1_op(i + 1, c1_cur)
                    if i < T:
                        # -- L2 matmuls: X2 = w2 @ c1(i) (fresh) --
                        X2 = x2pool.tile([128, W], F32, tag="X2", name="X2")
                        for m in range(KC):
                            for j_i, (k, part) in enumerate(
                                    ((0, "2h"), (0, "2l"),
                                     (1, "2h"), (1, "2l"))):
                                j = (k * 2 + m) * 128
                                nc.tensor.matmul(
                                    X2[:, m * BLK:(m + 1) * BLK],
                                    wt[part][:, j:j + 128],
                                    c1_cur[:, k * BLK:(k + 1) * BLK],
                                    start=(j_i == 0), stop=(j_i == 3))
                        c2n = npool.tile([128, W], FP16, tag="c2", name="c2")
                        nc.vector._custom_dve(op_count, out=c2n[:],
                                              in0=X2[:], in1=c2[:], s0=1.0)
                        c2 = c2n
                        if i + 1 < T:
                            c1_cur = c1_nxt
                        c2_by_t[i] = c2n
                    if i == 2 and b + 2 < NBLK:
                        stage_h1(b + 2)
                    if 3 <= i < T + 3:
                        t4 = i - 3
                        # -- L4 on counts: u(t) = w4 @ c3(t); the two
                        #    contract chunks run CONCURRENTLY in PE column
                        #    groups 0 and 32 (host sums the partition
                        #    slices during unsharding) --
                        c3t = c3_by_t.pop(t4)
                        o = opool.tile([48, BLK], F32, tag="o", name="o")
                        for k in range(KC):
                            nc.tensor.matmul(
                                o[32 * k:32 * k + 16, :],
                                w4h[:, k * 16:(k + 1) * 16],
                                c3t[:, k * BLK:(k + 1) * BLK],
                                start=True, stop=True,
                                tile_position=(0, 32 * k))
                        nc.scalar.copy(out=stage[:, t4 * BLK:(t4 + 1) * BLK],
                                       in_=o[:])
                        nc.sync.dma_start(out=out_d.ap()[t4, :, cs],
                                          in_=stage[:, t4 * BLK:(t4 + 1) * BLK])

    nc.compile()
    return nc


def _prep_inputs(x, w1, w2, w3, w4):
    """Host-side data prep: shard x, transpose/split weights (positive)."""
    def split(a):
        hi = a.astype(np.float16)
        lo = (a.astype(np.float32) - hi.astype(np.float32)).astype(np.float16)
        return np.ascontiguousarray(hi), np.ascontiguousarray(lo)

    def pack_256(wn):  # wn = w.T [256, 256] -> [128, 512] packed (k, m)
        out = np.empty((128, 512), np.float32)
        for k in range(2):
            for m in range(2):
                out[:, (k * 2 + m) * 128:(k * 2 + m + 1) * 128] = \
                    wn[k * 128:(k + 1) * 128, m * 128:(m + 1) * 128]
        return out

    def pack_16(wn):  # wn = w4.T [256, 16] -> [128, 32]
        out = np.empty((128, 32), np.float32)
        for k in range(2):
            out[:, k * 16:(k + 1) * 16] = wn[k * 128:(k + 1) * 128, :]
        return out

    w1hT, w1lT = split(w1.astype(np.float32).T)                  # [16,256]
    w2h, w2l = split(pack_256(w2.astype(np.float32).T))
    w3h, w3l = split(pack_256(w3.astype(np.float32).T))
    w4h = np.ascontiguousarray(
        pack_16(w4.astype(np.float32).T).astype(np.float16))
    w1S = np.zeros((48, 256), np.float16)
    w1S[0:16] = w1hT
    w1S[16:32] = w1lT
    w1S[32:48] = w1hT

    in_maps = []
    for c in range(NCORES):
        xT = np.ascontiguousarray(
            x[c * BL:(c + 1) * BL, :].astype(np.float32).T)      # [16, BL]
        xTh, xTl = split(xT)
        xS = np.zeros((48, xT.shape[1]), np.float16)
        xS[0:16] = xTh
        xS[16:32] = xTh
        xS[32:48] = xTl
        in_maps.append({
            "xS": xS, "w1S": w1S,
            "w2h": w2h, "w2l": w2l,
            "w3h": w3h, "w3l": w3l,
            "w4h": w4h,
        })
    return in_maps


def _get_nc():
    if "nc" not in _CACHE:
        _CACHE["nc"] = _build_nc()
    return _CACHE["nc"]


def run_sharded(x, w1, w2, w3, w4, **spmd_kwargs):
    """Run on 8 cores; returns (full_output, BassKernelResults)."""
    nc = _get_nc()
    in_maps = _prep_inputs(x, w1, w2, w3, w4)
    res = bass_utils.run_bass_kernel_spmd(
        nc, in_maps, core_ids=list(range(NCORES)), **spmd_kwargs)
    # per-core out holds the two col-group halves of u(t) = w4 @ c3(t)
    # in partition rows 0:16 and 32:48; out_t = u(t) - u(t-1).
    parts = []
    for r in res.results:
        u = (r["out"][:, 0:16].astype(np.float32)
             + r["out"][:, 32:48].astype(np.float32))
        u[1:] -= u[:-1].copy()
        parts.append(np.ascontiguousarray(u.transpose(0, 2, 1)))
    full = np.concatenate(parts, axis=1)
    return full, res


def kernel(x, w1, w2, w3, w4):
    full, _ = run_sharded(x, w1, w2, w3, w4)
    return full



# revision 11
# speedup vs baseline: 1.0076x; 1.0048x over previous
"""Trainium2 Bass kernel for a spiking MLP (IF neurons, soft reset) — v4.

Count-form reformulation. For a soft-reset IF neuron, with
X(t) = cumulative input and c(t) = cumulative spike count:

    spike(t)  =  X(t) >= 1 + c(t-1)
    c(t)      =  c(t-1) + spike(t)

and, critically, the next layer's cumulative input is itself a matmul
of the COUNTS:  X_{l+1}(t) = W_l @ c_l(t)  (counts are small exact ints
in fp16). So every timestep does FRESH matmuls on counts (start/stop
per step — no persistent PSUM integrators, no cross-step PSUM
serialization), and the per-layer elementwise work collapses into ONE
custom fused DVE op per layer:

    c' = c + ((X * s0) >= c + 1)        [SPIKE_COUNT_UPDATE]

(s0 = t+1 for layer 1, whose cumulative input is h1*(t+1); 1.0 for
layers 2/3). Layer 4 runs on the COUNTS (u(t) = w4 @ c3(t), the two
contract chunks concurrent in PE column groups 0/32); the exact
temporal diff out_t = u(t) - u(t-1) happens on host during unsharding.

Exactness: weights stream as fp16 hi/lo pairs (residual ~2^-22|w|) —
the 2-stream cost is the information floor (fp32r is 11-bit/TF32-like,
fp8 lo-terms flush to denormals, and the margin-noise budget is ~2e-6).

v4 over v3: (1) h1 = w1 @ x collapses its three contract-16 fp16 hi/lo
terms into ONE contract-48 matmul per m-chunk ([w1h; w1l; w1h] @
[xh; xh; xl] stacked along the contract dim); (2) the t-loop pipeline
is deepened — L3 runs 2 steps behind L2, L4 three behind, with DVE
issue order [c3(i-2), c1(i+1), c2(i)] so the single-buffered X3 bank
recycles without stalling the PE and the L2 group never waits on its
c1; (3) per-block xS DMA slices + block-0's first c1 reading h1
straight from PSUM shave the startup ramp. Measured: PE 95.3% busy
with ~5.5us true idle; ~525us vs the 532us v3 baseline.

PSUM: X2 double-buffered (4 banks) + X3 (2) + out double-buffered (2) = 8.
"""

import numpy as np

import concourse.bacc as bacc
import concourse.mybir as mybir
from concourse.tile import TileContext
from concourse import bass_utils

T = 16
B = 32768
NCORES = 8
BL = B // NCORES          # 4096 rows per core
BLK = 512                 # batch columns per block
D = 256                   # hidden width
KC = D // 128             # 2 feature chunks of 128
W = 2 * BLK               # paired-chunk tile width
NBLK = BL // BLK

F32 = mybir.dt.float32
FP16 = mybir.dt.float16

_CACHE = {}


def _register_count_op():
    """Register the fused spike-count-update DVE op:
        out = in1 + ((in0 * s0) >= in1 + 1)
    One pass over X and c producing the new count (spike folded in)."""
    from concourse import dve_ops
    from concourse.dve_spec import (
        Spec, Src0, Src1, One, lower as dve_lower, _has_src1)
    from concourse.dve_uop import DveOpSpec

    name = "SPIKE_COUNT_UPDATE_ANT"
    for op in dve_ops.OPS:
        if op.name == name:
            return op
    from concourse.dve_spec import C0
    spec = Spec(
        body=Src1 + ((Src0 * C0) >= (Src1 + One)),
        reference=lambda in0, in1, c0, c1, c2: in1
        + ((in0 * c0) >= (in1 + 1.0)),
    )
    row = dve_ops._CUSTOM_DVE_ROW_BASE + len(dve_ops.OPS)
    assert row < 0x20, "custom-DVE opcode rows exhausted"
    dve_ops._SUB_OPCODE_FOR_NAME[name] = row
    shas = {}
    for ver in ("v3", "v4"):
        try:
            uops = dve_lower(spec, ver=ver)
            shas[ver] = DveOpSpec(
                name=name, opcode=row, uops=uops,
                rd1_en=_has_src1(spec)).sha(ver)
        except Exception:
            pass
    op = dve_ops.DveOp(name, spec, subdim=False, uops_sha=shas)
    dve_ops.OPS.append(op)
    dve_ops.CUSTOM_DVE_SPECS[name] = spec
    return op


def _build_nc(bl=BL):
    op_count = _register_count_op()
    nc = bacc.Bacc("TRN2", target_bir_lowering=False, debug=False,
                   num_devices=NCORES)

    # h1 = w1h@xh + w1l@xh + w1h@xl: the three contract-16 terms stack
    # along the contract dim into ONE contract-48 matmul:
    # rows 0:16 = (w1h, xh), 16:32 = (w1l, xh), 32:48 = (w1h, xl).
    xS_d = nc.dram_tensor("xS", [48, bl], FP16, kind="ExternalInput")
    w1S_d = nc.dram_tensor("w1S", [48, D], FP16, kind="ExternalInput")
    # packed lhsT chunks, fp16 hi/lo:
    # [128, (k*2+m)*128 : ...] = w.T[k*128:(k+1)*128, m*128:(m+1)*128]
    w2h_d = nc.dram_tensor("w2h", [128, 512], FP16, kind="ExternalInput")
    w2l_d = nc.dram_tensor("w2l", [128, 512], FP16, kind="ExternalInput")
    w3h_d = nc.dram_tensor("w3h", [128, 512], FP16, kind="ExternalInput")
    w3l_d = nc.dram_tensor("w3l", [128, 512], FP16, kind="ExternalInput")
    # [128, k*16:(k+1)*16] = w4.T[k*128:(k+1)*128, :]
    w4h_d = nc.dram_tensor("w4h", [128, 32], FP16, kind="ExternalInput")
    out_d = nc.dram_tensor("out", [T, 48, bl], F32, kind="ExternalOutput")

    with TileContext(nc) as tc:
        with (
            tc.tile_pool(name="const", bufs=1) as cpool,
            tc.tile_pool(name="cnt", bufs=3) as npool,
            tc.tile_pool(name="stage", bufs=2) as stpool,
            tc.tile_pool(name="x2p", bufs=2, space="PSUM") as x2pool,
            tc.tile_pool(name="x3p", bufs=1, space="PSUM") as x3pool,
            tc.tile_pool(name="ops", bufs=2, space="PSUM") as opool,
        ):
            # ---- constants into SBUF ----
            w1S = cpool.tile([48, D], FP16, tag="w1S", name="w1S")
            nc.sync.dma_start(out=w1S[:], in_=w1S_d.ap())
            # per-block xS slices so stage_h1(0) starts after 1/8 of the DMA
            xS = cpool.tile([48, bl], FP16, tag="xS", name="xS")
            for _b in range(NBLK):
                _cs = slice(_b * BLK, (_b + 1) * BLK)
                nc.sync.dma_start(out=xS[:, _cs], in_=xS_d.ap()[:, _cs])
            wt = {}
            for name, dram in (("2h", w2h_d), ("2l", w2l_d),
                               ("3h", w3h_d), ("3l", w3l_d)):
                tl = cpool.tile([128, 512], FP16, tag=f"w{name}",
                                name=f"w{name}")
                nc.sync.dma_start(out=tl[:], in_=dram.ap())
                wt[name] = tl
            w4h = cpool.tile([128, 32], FP16, tag="w4h", name="w4h")
            nc.sync.dma_start(out=w4h[:], in_=w4h_d.ap())
            zero16 = cpool.tile([128, W], FP16, tag="zero16", name="zero16")
            nc.vector.memset(zero16[:], 0.0)

            # ---- h1 = w1 @ x for all blocks, staged through the X2 pool ----
            h1all = cpool.tile([128, NBLK * W], F32, tag="h1all",
                               name="h1all")
            def stage_h1(b):
                """h1 block b = (w1h+w1l) @ (xh+xl), minus the ~2^-22
                l*l term — one contract-48 matmul per m-chunk (the three
                contract-16 terms stacked along the contract dim)."""
                cs = slice(b * BLK, (b + 1) * BLK)
                hps = x2pool.tile([128, W], F32, tag="X2", name="X2")
                for m in range(KC):
                    ms = slice(m * 128, (m + 1) * 128)
                    nc.tensor.matmul(hps[:, m * BLK:(m + 1) * BLK],
                                     w1S[:, ms], xS[:, cs],
                                     start=True, stop=True)
                nc.scalar.copy(out=h1all[:, b * W:(b + 1) * W], in_=hps[:])
                return hps

            # block 0's h1 goes FIRST on the PE queue (it gates the whole
            # t-loop; it runs cold but ~2us earlier than behind the
            # warm-ups). The HAM warm-up matmuls then raise the PE clock
            # to 2.4 GHz while block 0's h1-copy and first count op run.
            h1ps0 = stage_h1(0)
            warm = x3pool.tile([128, W], F32, tag="X3", name="X3")
            for wi in range(10):
                nc.tensor.matmul(warm[:, 0:BLK], zero16[:, 0:128],
                                 zero16[:, 0:BLK], start=True, stop=True)
            if NBLK > 1:
                # block 1's h1 fills the PE gap while DVE/ACT finish
                # block 0's h1 copy and first count op
                stage_h1(1)
            # more warm-up matmuls AFTER the h1 stages: they fill the
            # ~2.2us PE gap while block 0's first count op runs on the
            # DVE (they sit ahead of L2(0) in the PE FIFO but finish
            # before c1(0) does).
            for wi in range(8):
                nc.tensor.matmul(warm[:, 0:BLK], zero16[:, 0:128],
                                 zero16[:, 0:BLK], start=True, stop=True)

            # Software pipeline (per block): L3 runs 2 steps behind L2, L4
            # runs 3 behind.  DVE issue order per iteration is
            # [c3(i-2), c1(i+1), c2(i)]: c3(i-2) reads the X3 produced by
            # the FIRST PE group of this iteration, so it finishes well
            # before the next iteration's L3 group needs the
            # (single-buffered) X3 bank back; c1 for the NEXT step is
            # computed an iteration ahead so the L2 group never waits on
            # it; and c2(i) — which must wait for this iteration's L2
            # matmuls — sits last so it never blocks ready DVE work.
            for b in range(NBLK):
                cs = slice(b * BLK, (b + 1) * BLK)
                h1b = h1all[:, b * W:(b + 1) * W]
                stage = stpool.tile([48, T * BLK], F32, tag="stage",
                                    name="stage")
                c2 = c3 = zero16

                def c1_op(step, c1_state, in0=None):
                    c1n = npool.tile([128, W], FP16, tag="c1", name="c1")
                    nc.vector._custom_dve(op_count, out=c1n[:],
                                          in0=h1b if in0 is None else in0,
                                          in1=c1_state[:],
                                          s0=float(step + 1))
                    return c1n

                # c1 for step 0, issued up front; block 0 reads h1 straight
                # from its PSUM staging tile instead of waiting for the ACT
                # copy into h1all.
                c1_cur = c1_op(0, zero16,
                               in0=h1ps0[:] if b == 0 else None)
                c2_by_t = {}
                c3_by_t = {}
                for i in range(T + 3):
                    if 2 <= i < T + 2:
                        t3 = i - 2
                        # -- L3 matmuls for step t3: X3 = w3 @ c2(t3) --
                        # For the LAST block's final two steps, borrow the
                        # (idle by then) X2 pool: the single-buffered X3
                        # bank otherwise serializes the tail drain
                        # L3(T-2) -> c3(T-2) -> L3(T-1) -> c3(T-1) -> L4.
                        tail = (b == NBLK - 1) and (t3 >= T - 2)
                        pool = x2pool if tail else x3pool
                        X3 = pool.tile([128, W], F32,
                                       tag="X2" if tail else "X3",
                                       name="X2" if tail else "X3")
                        c2t = c2_by_t.pop(t3)
                        for m in range(KC):
                            for j_i, (k, part) in enumerate(
                                    ((0, "3h"), (0, "3l"),
                                     (1, "3h"), (1, "3l"))):
                                j = (k * 2 + m) * 128
                                nc.tensor.matmul(
                                    X3[:, m * BLK:(m + 1) * BLK],
                                    wt[part][:, j:j + 128],
                                    c2t[:, k * BLK:(k + 1) * BLK],
                                    start=(j_i == 0), stop=(j_i == 3))
                        c3n = npool.tile([128, W], FP16, tag="c3", name="c3")
                        nc.vector._custom_dve(op_count, out=c3n[:],
                                              in0=X3[:], in1=c3[:], s0=1.0)
                        c3 = c3n
                        c3_by_t[t3] = c3n
                    if i + 1 < T:
                        # -- L1 count for the NEXT step, an iteration early
                        c1_nxt = c1_op(i + 1, c1_cur)
                    if i < T:
                        # -- L2 matmuls: X2 = w2 @ c1(i) (fresh) --
                        X2 = x2pool.tile([128, W], F32, tag="X2", name="X2")
                        for m in range(KC):
                            for j_i, (k, part) in enumerate(
                                    ((0, "2h"), (0, "2l"),
                                     (1, "2h"), (1, "2l"))):
                                j = (k * 2 + m) * 128
                                nc.tensor.matmul(
                                    X2[:, m * BLK:(m + 1) * BLK],
                                    wt[part][:, j:j + 128],
                                    c1_cur[:, k * BLK:(k + 1) * BLK],
                                    start=(j_i == 0), stop=(j_i == 3))
                        c2n = npool.tile([128, W], FP16, tag="c2", name="c2")
                        nc.vector._custom_dve(op_count, out=c2n[:],
                                              in0=X2[:], in1=c2[:], s0=1.0)
                        c2 = c2n
                        if i + 1 < T:
                            c1_cur = c1_nxt
                        c2_by_t[i] = c2n
                    if i == 2 and b + 2 < NBLK:
                        stage_h1(b + 2)
                    if 3 <= i < T + 3:
                        t4 = i - 3
                        # -- L4 on counts: u(t) = w4 @ c3(t); the two
                        #    contract chunks run CONCURRENTLY in PE column
                        #    groups 0 and 32 (host sums the partition
                        #    slices during unsharding) --
                        c3t = c3_by_t.pop(t4)
                        o = opool.tile([48, BLK], F32, tag="o", name="o")
                        for k in range(KC):
                            nc.tensor.matmul(
                                o[32 * k:32 * k + 16, :],
                                w4h[:, k * 16:(k + 1) * 16],
                                c3t[:, k * BLK:(k + 1) * BLK],
                                start=True, stop=True,
                                tile_position=(0, 32 * k))
                        nc.scalar.copy(out=stage[:, t4 * BLK:(t4 + 1) * BLK],
                                       in_=o[:])
                        nc.sync.dma_start(out=out_d.ap()[t4, :, cs],
                                          in_=stage[:, t4 * BLK:(t4 + 1) * BLK])

    nc.compile()
    return nc


def _prep_inputs(x, w1, w2, w3, w4):
    """Host-side data prep: shard x, transpose/split weights (positive)."""
    def split(a):
        hi = a.astype(np.float16)
        lo = (a.astype(np.float32) - hi.astype(np.float32)).astype(np.float16)
        return np.ascontiguousarray(hi), np.ascontiguousarray(lo)

    def pack_256(wn):  # wn = w.T [256, 256] -> [128, 512] packed (k, m)
        out = np.empty((128, 512), np.float32)
        for k in range(2):
            for m in range(2):
                out[:, (k * 2 + m) * 128:(k * 2 + m + 1) * 128] = \
                    wn[k * 128:(k + 1) * 128, m * 128:(m + 1) * 128]
        return out

    def pack_16(wn):  # wn = w4.T [256, 16] -> [128, 32]
        out = np.empty((128, 32), np.float32)
        for k in range(2):
            out[:, k * 16:(k + 1) * 16] = wn[k * 128:(k + 1) * 128, :]
        return out

    w1hT, w1lT = split(w1.astype(np.float32).T)                  # [16,256]
    w2h, w2l = split(pack_256(w2.astype(np.float32).T))
    w3h, w3l = split(pack_256(w3.astype(np.float32).T))
    w4h = np.ascontiguousarray(
        pack_16(w4.astype(np.float32).T).astype(np.float16))
    w1S = np.zeros((48, 256), np.float16)
    w1S[0:16] = w1hT
    w1S[16:32] = w1lT
    w1S[32:48] = w1hT

    in_maps = []
    for c in range(NCORES):
        xT = np.ascontiguousarray(
            x[c * BL:(c + 1) * BL, :].astype(np.float32).T)      # [16, BL]
        xTh, xTl = split(xT)
        xS = np.zeros((48, xT.shape[1]), np.float16)
        xS[0:16] = xTh
        xS[16:32] = xTh
        xS[32:48] = xTl
        in_maps.append({
            "xS": xS, "w1S": w1S,
            "w2h": w2h, "w2l": w2l,
            "w3h": w3h, "w3l": w3l,
            "w4h": w4h,
        })
    return in_maps


def _get_nc():
    if "nc" not in _CACHE:
        _CACHE["nc"] = _build_nc()
    return _CACHE["nc"]


def run_sharded(x, w1, w2, w3, w4, **spmd_kwargs):
    """Run on 8 cores; returns (full_output, BassKernelResults)."""
    nc = _get_nc()
    in_maps = _prep_inputs(x, w1, w2, w3, w4)
    res = bass_utils.run_bass_kernel_spmd(
        nc, in_maps, core_ids=list(range(NCORES)), **spmd_kwargs)
    # per-core out holds the two col-group halves of u(t) = w4 @ c3(t)
    # in partition rows 0:16 and 32:48; out_t = u(t) - u(t-1).
    parts = []
    for r in res.results:
        u = (r["out"][:, 0:16].astype(np.float32)
             + r["out"][:, 32:48].astype(np.float32))
        u[1:] -= u[:-1].copy()
        parts.append(np.ascontiguousarray(u.transpose(0, 2, 1)))
    full = np.concatenate(parts, axis=1)
    return full, res


def kernel(x, w1, w2, w3, w4):
    full, _ = run_sharded(x, w1, w2, w3, w4)
    return full



# revision 15
# speedup vs baseline: 1.0294x; 1.0216x over previous
"""Trainium2 Bass kernel for a spiking MLP (IF neurons, soft reset) — v4.

Count-form reformulation. For a soft-reset IF neuron, with
X(t) = cumulative input and c(t) = cumulative spike count:

    spike(t)  =  X(t) >= 1 + c(t-1)
    c(t)      =  c(t-1) + spike(t)

and, critically, the next layer's cumulative input is itself a matmul
of the COUNTS:  X_{l+1}(t) = W_l @ c_l(t)  (counts are small exact ints
in fp16). So every timestep does FRESH matmuls on counts (start/stop
per step — no persistent PSUM integrators, no cross-step PSUM
serialization), and the per-layer elementwise work collapses into ONE
custom fused DVE op per layer:

    c' = c + ((X * s0) >= c + 1)        [SPIKE_COUNT_UPDATE]

(s0 = t+1 for layer 1, whose cumulative input is h1*(t+1); 1.0 for
layers 2/3). Layer 4 runs on the COUNTS (u(t) = w4 @ c3(t), the two
contract chunks concurrent in PE column groups 0/32); the exact
temporal diff out_t = u(t) - u(t-1) happens on host during unsharding.

Exactness: weights stream as fp16 hi/lo pairs (residual ~2^-22|w|) —
the 2-stream cost is the information floor (fp32r is 11-bit/TF32-like,
fp8 lo-terms flush to denormals, and the margin-noise budget is ~2e-6).

v4 over v3: (1) h1 = w1 @ x collapses its three contract-16 fp16 hi/lo
terms into ONE contract-48 matmul per m-chunk ([w1h; w1l; w1h] @
[xh; xh; xl] stacked along the contract dim); (2) the t-loop pipeline
is deepened — L3 runs 2 steps behind L2, L4 three behind, with DVE
issue order [c3(i-2), c1(i+1), c2(i)] so the single-buffered X3 bank
recycles without stalling the PE and the L2 group never waits on its
c1; (3) per-block xS DMA slices + block-0's first c1 reading h1
straight from PSUM shave the startup ramp. Measured: PE 95.3% busy
with ~5.5us true idle; ~525us vs the 532us v3 baseline.

PSUM: X2 double-buffered (4 banks) + X3 (2) + out double-buffered (2) = 8.
"""

import numpy as np

import concourse.bacc as bacc
import concourse.mybir as mybir
from concourse.tile import TileContext
from concourse import bass_utils

T = 16
B = 32768
NCORES = 8
BL = B // NCORES          # 4096 rows per core
BLK = 512                 # batch columns per block
D = 256                   # hidden width
KC = D // 128             # 2 feature chunks of 128
W = 2 * BLK               # paired-chunk tile width
NBLK = BL // BLK

F32 = mybir.dt.float32
FP16 = mybir.dt.float16

_CACHE = {}


def _register_count_op():
    """Register the fused spike-count-update DVE op:
        out = in1 + ((in0 * s0) >= in1 + 1)
    One pass over X and c producing the new count (spike folded in)."""
    from concourse import dve_ops
    from concourse.dve_spec import (
        Spec, Src0, Src1, One, lower as dve_lower, _has_src1)
    from concourse.dve_uop import DveOpSpec

    name = "SPIKE_COUNT_UPDATE_ANT"
    for op in dve_ops.OPS:
        if op.name == name:
            return op
    from concourse.dve_spec import C0
    spec = Spec(
        body=Src1 + ((Src0 * C0) >= (Src1 + One)),
        reference=lambda in0, in1, c0, c1, c2: in1
        + ((in0 * c0) >= (in1 + 1.0)),
    )
    row = dve_ops._CUSTOM_DVE_ROW_BASE + len(dve_ops.OPS)
    assert row < 0x20, "custom-DVE opcode rows exhausted"
    dve_ops._SUB_OPCODE_FOR_NAME[name] = row
    shas = {}
    for ver in ("v3", "v4"):
        try:
            uops = dve_lower(spec, ver=ver)
            shas[ver] = DveOpSpec(
                name=name, opcode=row, uops=uops,
                rd1_en=_has_src1(spec)).sha(ver)
        except Exception:
            pass
    op = dve_ops.DveOp(name, spec, subdim=False, uops_sha=shas)
    dve_ops.OPS.append(op)
    dve_ops.CUSTOM_DVE_SPECS[name] = spec
    return op


def _build_nc(bl=BL):
    op_count = _register_count_op()
    nc = bacc.Bacc("TRN2", target_bir_lowering=False, debug=False,
                   num_devices=NCORES)

    # h1 = w1h@xh + w1l@xh + w1h@xl: the three contract-16 terms stack
    # along the contract dim into ONE contract-48 matmul:
    # rows 0:16 = (w1h, xh), 16:32 = (w1l, xh), 32:48 = (w1h, xl).
    xS_d = nc.dram_tensor("xS", [48, bl], FP16, kind="ExternalInput")
    w1S_d = nc.dram_tensor("w1S", [48, D], FP16, kind="ExternalInput")
    # packed lhsT chunks, fp16 hi/lo:
    # [128, (k*2+m)*128 : ...] = w.T[k*128:(k+1)*128, m*128:(m+1)*128]
    w2h_d = nc.dram_tensor("w2h", [128, 512], FP16, kind="ExternalInput")
    w2l_d = nc.dram_tensor("w2l", [128, 512], FP16, kind="ExternalInput")
    w3h_d = nc.dram_tensor("w3h", [128, 512], FP16, kind="ExternalInput")
    w3l_d = nc.dram_tensor("w3l", [128, 512], FP16, kind="ExternalInput")
    # [128, k*16:(k+1)*16] = w4.T[k*128:(k+1)*128, :]
    w4h_d = nc.dram_tensor("w4h", [128, 32], FP16, kind="ExternalInput")
    # two timesteps per slab: rows 0:16/32:48 = u(2j) halves, rows
    # 64:80/96:112 = u(2j+1) halves (host sums/diffs during unshard)
    out_d = nc.dram_tensor("out", [T // 2, 128, bl], F32,
                           kind="ExternalOutput")

    with TileContext(nc) as tc:
        with (
            tc.tile_pool(name="const", bufs=1) as cpool,
            tc.tile_pool(name="cnt", bufs=3) as npool,
            tc.tile_pool(name="stage", bufs=2) as stpool,
            tc.tile_pool(name="x2p", bufs=2, space="PSUM") as x2pool,
            tc.tile_pool(name="x3p", bufs=1, space="PSUM") as x3pool,
            tc.tile_pool(name="ops", bufs=2, space="PSUM") as opool,
        ):
            # ---- constants into SBUF ----
            w1S = cpool.tile([48, D], FP16, tag="w1S", name="w1S")
            nc.sync.dma_start(out=w1S[:], in_=w1S_d.ap())
            # per-block xS slices so stage_h1(0) starts after 1/8 of the DMA
            xS = cpool.tile([48, bl], FP16, tag="xS", name="xS")
            for _b in range(NBLK):
                _cs = slice(_b * BLK, (_b + 1) * BLK)
                nc.sync.dma_start(out=xS[:, _cs], in_=xS_d.ap()[:, _cs])
            wt = {}
            for name, dram in (("2h", w2h_d), ("2l", w2l_d),
                               ("3h", w3h_d), ("3l", w3l_d)):
                tl = cpool.tile([128, 512], FP16, tag=f"w{name}",
                                name=f"w{name}")
                nc.sync.dma_start(out=tl[:], in_=dram.ap())
                wt[name] = tl
            w4h = cpool.tile([128, 32], FP16, tag="w4h", name="w4h")
            nc.sync.dma_start(out=w4h[:], in_=w4h_d.ap())
            zero16 = cpool.tile([128, W], FP16, tag="zero16", name="zero16")
            nc.vector.memset(zero16[:], 0.0)

            # ---- h1 = w1 @ x for all blocks, staged through the X2 pool ----
            h1all = cpool.tile([128, NBLK * W], F32, tag="h1all",
                               name="h1all")
            def stage_h1(b):
                """h1 block b = (w1h+w1l) @ (xh+xl), minus the ~2^-22
                l*l term — one contract-48 matmul per m-chunk (the three
                contract-16 terms stacked along the contract dim)."""
                cs = slice(b * BLK, (b + 1) * BLK)
                hps = x2pool.tile([128, W], F32, tag="X2", name="X2")
                for m in range(KC):
                    ms = slice(m * 128, (m + 1) * 128)
                    nc.tensor.matmul(hps[:, m * BLK:(m + 1) * BLK],
                                     w1S[:, ms], xS[:, cs],
                                     start=True, stop=True)
                nc.scalar.copy(out=h1all[:, b * W:(b + 1) * W], in_=hps[:])
                return hps

            # block 0's h1 goes FIRST on the PE queue (it gates the whole
            # t-loop; it runs cold but ~2us earlier than behind the
            # warm-ups). The HAM warm-up matmuls then raise the PE clock
            # to 2.4 GHz while block 0's h1-copy and first count op run.
            h1ps0 = stage_h1(0)
            warm = x3pool.tile([128, W], F32, tag="X3", name="X3")
            for wi in range(10):
                nc.tensor.matmul(warm[:, 0:BLK], zero16[:, 0:128],
                                 zero16[:, 0:BLK], start=True, stop=True)
            if NBLK > 1:
                # block 1's h1 fills the PE gap while DVE/ACT finish
                # block 0's h1 copy and first count op
                stage_h1(1)
            # more warm-up matmuls AFTER the h1 stages: they fill the
            # ~2.2us PE gap while block 0's first count op runs on the
            # DVE (they sit ahead of L2(0) in the PE FIFO but finish
            # before c1(0) does).
            for wi in range(8):
                nc.tensor.matmul(warm[:, 0:BLK], zero16[:, 0:128],
                                 zero16[:, 0:BLK], start=True, stop=True)

            # Software pipeline (per block): L3 runs 2 steps behind L2, L4
            # runs 3 behind.  DVE issue order per iteration is
            # [c3(i-2), c1(i+1), c2(i)]: c3(i-2) reads the X3 produced by
            # the FIRST PE group of this iteration, so it finishes well
            # before the next iteration's L3 group needs the
            # (single-buffered) X3 bank back; c1 for the NEXT step is
            # computed an iteration ahead so the L2 group never waits on
            # it; and c2(i) — which must wait for this iteration's L2
            # matmuls — sits last so it never blocks ready DVE work.
            for b in range(NBLK):
                cs = slice(b * BLK, (b + 1) * BLK)
                h1b = h1all[:, b * W:(b + 1) * W]
                stage = stpool.tile([128, (T // 2) * BLK], F32, tag="stage",
                                    name="stage")
                c2 = c3 = zero16

                def c1_op(step, c1_state, in0=None):
                    c1n = npool.tile([128, W], FP16, tag="c1", name="c1")
                    nc.vector._custom_dve(op_count, out=c1n[:],
                                          in0=h1b if in0 is None else in0,
                                          in1=c1_state[:],
                                          s0=float(step + 1))
                    return c1n

                # c1 for step 0, issued up front; block 0 reads h1 straight
                # from its PSUM staging tile instead of waiting for the ACT
                # copy into h1all.
                c1_cur = c1_op(0, zero16,
                               in0=h1ps0[:] if b == 0 else None)
                c2_by_t = {}
                c3_by_t = {}
                for i in range(T + 3):
                    if 2 <= i < T + 2:
                        t3 = i - 2
                        # -- L3 matmuls for step t3: X3 = w3 @ c2(t3) --
                        # For the LAST block's final two steps, borrow the
                        # (idle by then) X2 pool: the single-buffered X3
                        # bank otherwise serializes the tail drain
                        # L3(T-2) -> c3(T-2) -> L3(T-1) -> c3(T-1) -> L4.
                        tail = (b == NBLK - 1) and (t3 >= T - 2)
                        pool = x2pool if tail else x3pool
                        X3 = pool.tile([128, W], F32,
                                       tag="X2" if tail else "X3",
                                       name="X2" if tail else "X3")
                        c2t = c2_by_t.pop(t3)
                        for m in range(KC):
                            for j_i, (k, part) in enumerate(
                                    ((0, "3h"), (0, "3l"),
                                     (1, "3h"), (1, "3l"))):
                                j = (k * 2 + m) * 128
                                nc.tensor.matmul(
                                    X3[:, m * BLK:(m + 1) * BLK],
                                    wt[part][:, j:j + 128],
                                    c2t[:, k * BLK:(k + 1) * BLK],
                                    start=(j_i == 0), stop=(j_i == 3))
                        c3n = npool.tile([128, W], FP16, tag="c3", name="c3")
                        nc.vector._custom_dve(op_count, out=c3n[:],
                                              in0=X3[:], in1=c3[:], s0=1.0)
                        c3 = c3n
                        c3_by_t[t3] = c3n
                    if i + 1 < T:
                        # -- L1 count for the NEXT step, an iteration early
                        c1_nxt = c1_op(i + 1, c1_cur)
                    if i < T:
                        # -- L2 matmuls: X2 = w2 @ c1(i) (fresh) --
                        X2 = x2pool.tile([128, W], F32, tag="X2", name="X2")
                        for m in range(KC):
                            for j_i, (k, part) in enumerate(
                                    ((0, "2h"), (0, "2l"),
                                     (1, "2h"), (1, "2l"))):
                                j = (k * 2 + m) * 128
                                nc.tensor.matmul(
                                    X2[:, m * BLK:(m + 1) * BLK],
                                    wt[part][:, j:j + 128],
                                    c1_cur[:, k * BLK:(k + 1) * BLK],
                                    start=(j_i == 0), stop=(j_i == 3))
                        c2n = npool.tile([128, W], FP16, tag="c2", name="c2")
                        nc.vector._custom_dve(op_count, out=c2n[:],
                                              in0=X2[:], in1=c2[:], s0=1.0)
                        c2 = c2n
                        if i + 1 < T:
                            c1_cur = c1_nxt
                        c2_by_t[i] = c2n
                    if i == 2 and b + 2 < NBLK:
                        stage_h1(b + 2)
                    if i >= 3 and i % 2 == 1 and i - 3 < T:
                        t4 = i - 3
                        # -- L4 on counts for TWO steps: u(t) = w4 @ c3(t)
                        #    for t in {t4, t4+1}. All four 16-wide contract
                        #    chunks run CONCURRENTLY in PE column groups
                        #    0/32/64/96 (disjoint PSUM partition slices;
                        #    host sums/diffs during unsharding) --
                        c3a = c3_by_t.pop(t4)
                        c3b = c3_by_t.pop(t4 + 1)
                        o = opool.tile([128, BLK], F32, tag="o", name="o")
                        for idx, (c3t, k) in enumerate(
                                ((c3a, 0), (c3a, 1), (c3b, 0), (c3b, 1))):
                            nc.tensor.matmul(
                                o[32 * idx:32 * idx + 16, :],
                                w4h[:, k * 16:(k + 1) * 16],
                                c3t[:, k * BLK:(k + 1) * BLK],
                                start=True, stop=True,
                                tile_position=(0, 32 * idx))
                        jp = t4 // 2
                        nc.scalar.copy(out=stage[:, jp * BLK:(jp + 1) * BLK],
                                       in_=o[:])
                        nc.sync.dma_start(out=out_d.ap()[jp, :, cs],
                                          in_=stage[:, jp * BLK:(jp + 1) * BLK])

    nc.compile()
    return nc


def _prep_inputs(x, w1, w2, w3, w4):
    """Host-side data prep: shard x, transpose/split weights (positive)."""
    def split(a):
        hi = a.astype(np.float16)
        lo = (a.astype(np.float32) - hi.astype(np.float32)).astype(np.float16)
        return np.ascontiguousarray(hi), np.ascontiguousarray(lo)

    def pack_256(wn):  # wn = w.T [256, 256] -> [128, 512] packed (k, m)
        out = np.empty((128, 512), np.float32)
        for k in range(2):
            for m in range(2):
                out[:, (k * 2 + m) * 128:(k * 2 + m + 1) * 128] = \
                    wn[k * 128:(k + 1) * 128, m * 128:(m + 1) * 128]
        return out

    def pack_16(wn):  # wn = w4.T [256, 16] -> [128, 32]
        out = np.empty((128, 32), np.float32)
        for k in range(2):
            out[:, k * 16:(k + 1) * 16] = wn[k * 128:(k + 1) * 128, :]
        return out

    w1hT, w1lT = split(w1.astype(np.float32).T)                  # [16,256]
    w2h, w2l = split(pack_256(w2.astype(np.float32).T))
    w3h, w3l = split(pack_256(w3.astype(np.float32).T))
    w4h = np.ascontiguousarray(
        pack_16(w4.astype(np.float32).T).astype(np.float16))
    w1S = np.zeros((48, 256), np.float16)
    w1S[0:16] = w1hT
    w1S[16:32] = w1lT
    w1S[32:48] = w1hT

    in_maps = []
    for c in range(NCORES):
        xT = np.ascontiguousarray(
            x[c * BL:(c + 1) * BL, :].astype(np.float32).T)      # [16, BL]
        xTh, xTl = split(xT)
        xS = np.zeros((48, xT.shape[1]), np.float16)
        xS[0:16] = xTh
        xS[16:32] = xTh
        xS[32:48] = xTl
        in_maps.append({
            "xS": xS, "w1S": w1S,
            "w2h": w2h, "w2l": w2l,
            "w3h": w3h, "w3l": w3l,
            "w4h": w4h,
        })
    return in_maps


def _get_nc():
    if "nc" not in _CACHE:
        _CACHE["nc"] = _build_nc()
    return _CACHE["nc"]


def run_sharded(x, w1, w2, w3, w4, **spmd_kwargs):
    """Run on 8 cores; returns (full_output, BassKernelResults)."""
    nc = _get_nc()
    in_maps = _prep_inputs(x, w1, w2, w3, w4)
    res = bass_utils.run_bass_kernel_spmd(
        nc, in_maps, core_ids=list(range(NCORES)), **spmd_kwargs)
    # per-core out [T/2, 128, bl]: rows 0:16/32:48 are the two contract
    # halves of u(2j), rows 64:80/96:112 of u(2j+1); out_t = u(t)-u(t-1).
    parts = []
    for r in res.results:
        o = r["out"].astype(np.float32)
        bl = o.shape[2]
        u = np.empty((T, 16, bl), np.float32)
        u[0::2] = o[:, 0:16] + o[:, 32:48]
        u[1::2] = o[:, 64:80] + o[:, 96:112]
        u[1:] -= u[:-1].copy()
        parts.append(np.ascontiguousarray(u.transpose(0, 2, 1)))
    full = np.concatenate(parts, axis=1)
    return full, res


def kernel(x, w1, w2, w3, w4):
    full, _ = run_sharded(x, w1, w2, w3, w4)
    return full

